# revision 51
# baseline (speedup 1.0000x reference)
"""HeightmapNormalsLoss TRN2 kernel, v3.

Data-parallel over 8 NeuronCores: 4 image-pairs per core.

Per image: Sobel gx/gy via TensorEngine band matmuls in f32r (vertical
bands stationary, horizontal taps as shifted column streams of an
edge-padded f32 tile; weights pre-scaled by sqrt(63)), then:

  q   = gx'^2 + gy'^2 + 1          (DVE custom fused op, = 63*s+1)
  u   = 1/q                        (DVE RECIPROCAL_APPROX_FAST; keeping
                                    it off ACT unchokes the extract path)
  inv = sqrt(16/63 * u)            (ACT Sqrt)
  n_z = sqrt(64/63 * u - 1/63)     (ACT Sqrt, written directly)
  n_x = gx'*inv, n_y = gy'*inv     (DVE in-place muls, fp16 2x)
  acc += sum |n_gen - n_tgt|       (pipelined pairs: DVE sub + ACT
                                    Abs-accum; final pair mostly via the
                                    DVE fused sub+abs+reduce op, since
                                    ACT is the straggler in the tail)

Row tiling: 4 full 128-row tiles per image cover rows 0..504; the last
7 rows of all 8 images are packed (via staging + GpSimd-issued SBUF DMA)
into one 2-page tile whose chain runs once in the pipeline tail. PSUM
extraction on ACT Copy (rt3 on DVE for balance); a software pipeline
runs pair p's matmul/extract ahead of pair p-1's sqrt/mul/reduce tail so
every engine queue stays fed. Per-core output: [128, 17] f32 partial
sums; host reduces and divides.
"""
import sys

sys.path.insert(0, "/opt/trn_rl_repo")

import numpy as np

H = W = 512
N_CORES = 8
PAIRS_PER_CORE = 4
TOTAL_B = 32
NT = 4  # full row tiles per image (the 7-row tail is packed separately)
S63 = float(np.sqrt(63.0))

# (out_row_start, M, in_row_start, variant); K = 128 for all tiles.
ROW_TILES = [
    (0, 127, 0, 0),
    (127, 126, 126, 1),
    (253, 126, 252, 1),
    (379, 126, 378, 1),
]
T4 = (505, 7, 384, 2)
N_ACC_COLS = PAIRS_PER_CORE * NT + 1  # 17 (last col = packed 7-row tail)


def _build_bands_np():
    """[128, 12*128] f32: blocks (band*3 + variant); bands sv, -sv, dv, 2dv,
    all scaled by sqrt(63). Variant 2 sits at partitions 120..127 (tile
    loaded from row 384 so K=128 stays in bounds)."""
    mats = {}
    for v, M in ((0, 127), (1, 126), (2, 7)):
        sv = np.zeros((128, 128), np.float64)
        dv = np.zeros((128, 128), np.float64)
        if v == 0:  # first tile: in-row p = image row p; m=0 clamps row -1 -> 0
            sv[0, 0], sv[1, 0] = 3.0, 1.0
            dv[0, 0], dv[1, 0] = 1.0, -1.0
            for m in range(1, M):
                sv[m - 1, m], sv[m, m], sv[m + 1, m] = 1.0, 2.0, 1.0
                dv[m - 1, m], dv[m + 1, m] = 1.0, -1.0
        elif v == 1:  # mid tiles: out r0+m taps partitions m, m+1, m+2
            for m in range(M):
                sv[m, m], sv[m + 1, m], sv[m + 2, m] = 1.0, 2.0, 1.0
                dv[m, m], dv[m + 2, m] = 1.0, -1.0
        else:  # last tile: rows 505..511 from partitions 120..127; clamp row 512
            for m in range(M - 1):
                sv[120 + m, m], sv[121 + m, m], sv[122 + m, m] = 1.0, 2.0, 1.0
                dv[120 + m, m], dv[122 + m, m] = 1.0, -1.0
            m = M - 1
            sv[126, m], sv[127, m] = 1.0, 3.0
            dv[126, m], dv[127, m] = 1.0, -1.0
        mats[(0, v)] = sv
        mats[(1, v)] = -sv
        mats[(2, v)] = dv
        mats[(3, v)] = 2.0 * dv
    w = np.zeros((128, 12 * 128), np.float64)
    for b in range(4):
        for v in range(3):
            w[:, (b * 3 + v) * 128 : (b * 3 + v + 1) * 128] = mats[(b, v)]
    return (w * S63).astype(np.float32)


def _register_ops():
    """Register fused custom DVE ops (runtime append to dve_ops.OPS, sha
    computed self-consistently):
      SUMSQ1_ANT:  out = in0^2 + in1^2 + 1
      SUBABS_ANT:  out = |in0 - in1|, accum_out = sum(out)
    """
    import concourse.dve_ops as dve_ops
    from concourse.dve_spec import Spec, Src0, Src1, One, Zero, maxx, lower
    from concourse.dve_uop import DveOpSpec
    from operator import add

    def reg(name, spec):
        for o in dve_ops.OPS:
            if o.name == name:
                return o
        row = dve_ops._CUSTOM_DVE_ROW_BASE + len(dve_ops.OPS)
        shas = {}
        for ver in ("v3", "v4"):
            uops = lower(spec, ver=ver)
            shas[ver] = DveOpSpec(
                name=name, opcode=row, uops=uops, rd1_en=True
            ).sha(ver)
        op = dve_ops.DveOp(name, spec, subdim=False, uops_sha=shas)
        dve_ops.OPS.append(op)
        dve_ops._SUB_OPCODE_FOR_NAME[name] = row
        dve_ops.CUSTOM_DVE_SPECS[name] = spec
        return op

    def sumsq_ref(in0, in1, s0, s1, imm2):
        return (
            in0.astype(np.float32) ** 2 + in1.astype(np.float32) ** 2 + 1.0
        ).astype(np.float32)

    def subabs_ref(in0, in1, s0, s1, imm2):
        b = np.abs(in0.astype(np.float32) - in1.astype(np.float32)).astype(
            np.float32
        )
        return b, b.reshape(b.shape[0], -1).sum(axis=-1, keepdims=True)

    sumsq = reg(
        "SUMSQ1_ANT", Spec(body=Src0 * Src0 + Src1 * Src1 + One, reference=sumsq_ref)
    )
    subabs = reg(
        "SUBABS_ANT",
        Spec(
            body=maxx(Src0 - Src1, Src1 - Src0),
            accum=add,
            accum_init=Zero,
            reference=subabs_ref,
        ),
    )
    return sumsq, subabs


def _act_recip(nc, out, in_):
    """ACT Reciprocal via direct InstActivation (wrapper blocks it for
    accuracy; 2e-2 tolerance here makes the table fine)."""
    from concourse import mybir

    sc = nc.scalar
    ins = [sc.lower_ap(in_)]
    for arg in (0.0, 1.0, 0.0):  # bias, scale, alpha
        ins.append(mybir.ImmediateValue(dtype=mybir.dt.float32, value=float(arg)))
    return sc.add_instruction(
        mybir.InstActivation(
            name=nc.get_next_instruction_name(),
            func=mybir.ActivationFunctionType.Reciprocal,
            ins=ins,
            outs=[sc.lower_ap(out)],
        )
    )


def _kernel_body(tc, gen_d, tgt_d, w_d, acc_d, sumsq_op, subabs_op):
    from contextlib import ExitStack
    from concourse import mybir
    from concourse.dve_ops import RECIPROCAL_APPROX_FAST, RECIP_APPROX_FAST_CONSTS

    nc = tc.nc
    AF = mybir.ActivationFunctionType
    OP = mybir.AluOpType
    f32 = mybir.dt.float32
    f32r = mybir.dt.float32r
    f16 = mybir.dt.float16

    with ExitStack() as ctx:
        persist = ctx.enter_context(tc.tile_pool(name="persist", bufs=1))
        xp_pool = ctx.enter_context(tc.tile_pool(name="xp", bufs=4))
        ps_pool = ctx.enter_context(tc.tile_pool(name="ps", bufs=2, space="PSUM"))
        gq_pool = ctx.enter_context(tc.tile_pool(name="gq", bufs=2))
        q_pool = ctx.enter_context(tc.tile_pool(name="q", bufs=2))
        iv_pool = ctx.enter_context(tc.tile_pool(name="iv", bufs=2))
        sc_pool = ctx.enter_context(tc.tile_pool(name="sc", bufs=1))
        st_pool = ctx.enter_context(tc.tile_pool(name="st", bufs=2))

        wt = persist.tile([128, 12 * 128], f32r)
        nc.sync.dma_start(wt[:], w_d[:])
        accbuf = persist.tile([128, N_ACC_COLS], f32)
        nc.vector.memset(accbuf[:], 0.0)
        bias0 = persist.tile([128, 1], f32)
        nc.vector.memset(bias0[:], 0.0)
        bias_nz = persist.tile([128, 1], f32)
        nc.vector.memset(bias_nz[:], -1.0 / 63.0)
        # packed tail tile: page 0 = gen, page 1 = tgt; pair p at
        # partitions 7p..7p+6 (written by DMA, which has no partition-
        # alignment restriction; compute reads all start at partition 0)
        tz = persist.tile([128, 2, 3 * W], f16)
        qz = persist.tile([128, 2, W], f16)
        ziv = persist.tile([128, 2, W], f16)

        def w_sl(band, variant, M):
            blk = (band * 3 + variant) * 128
            return wt[0:128, blk : blk + M]

        def five_matmuls(pt, x, rt, v, M):
            """x: [128, 2img, NT+1, W+2]; pt: [128, 2img, 2W] psum; the two
            images' five streams interleave per stationary (one LDW each)."""
            sv, nsv = w_sl(0, v, M), w_sl(1, v, M)
            dv, dv2 = w_sl(2, v, M), w_sl(3, v, M)
            for i in (0, 1):
                gx = pt[0:M, i, 0:W]
                nc.tensor.matmul(
                    gx, sv, x[:, i, rt, 0:W], start=True, stop=False
                )
                nc.tensor.matmul(
                    gx, nsv, x[:, i, rt, 2 : W + 2], start=False, stop=True
                )
            for i in (0, 1):
                gy = pt[0:M, i, W : 2 * W]
                nc.tensor.matmul(
                    gy, dv, x[:, i, rt, 0:W], start=True, stop=False
                )
                nc.tensor.matmul(
                    gy, dv, x[:, i, rt, 2 : W + 2], start=False, stop=False
                )
                nc.tensor.matmul(
                    gy, dv2, x[:, i, rt, 1 : W + 1], start=False, stop=True
                )

        def stage_a1(pair, cb=None):
            """DMA, matmuls, extract -> gq (4 row tiles) + tz (7-row tail).
            cb(rt), if given, issues one piece of the previous pair's sqrt
            work after each row-tile extract (keeps ACT from blocking PE)."""
            xp = xp_pool.tile([128, 2, NT + 1, W + 2], f32r, tag="xp")
            for rt, (r0, M, i0, v) in enumerate(ROW_TILES + [T4]):
                for imi, src in ((0, gen_d), (1, tgt_d)):
                    nc.sync.dma_start(
                        xp[0:128, imi, rt, 1 : W + 1], src[pair, i0 : i0 + 128, :]
                    )
                    if pair == 0 and rt == 0:
                        # only the very first matmuls need an early start:
                        # pad rt0 alone, batch everything else
                        nc.vector.tensor_copy(
                            xp[:, imi, rt, 0:1], xp[:, imi, rt, 1:2]
                        )
                        nc.vector.tensor_copy(
                            xp[:, imi, rt, W + 1 : W + 2],
                            xp[:, imi, rt, W : W + 1],
                        )
            lo = 1 if pair == 0 else 0
            for imi in (0, 1):
                nc.vector.tensor_copy(
                    xp[:, imi, lo:, 0:1], xp[:, imi, lo:, 1:2]
                )
                nc.vector.tensor_copy(
                    xp[:, imi, lo:, W + 1 : W + 2], xp[:, imi, lo:, W : W + 1]
                )

            gq = gq_pool.tile([128, 2 * NT, 3 * W], f16, tag="gq")
            q = q_pool.tile([128, 2 * NT, W], f16, tag="q")
            x = xp[0:128, :, :, :]
            for rt, (r0, M, i0, v) in enumerate(ROW_TILES):
                pt = ps_pool.tile([128, 2, 2 * W], f32, tag="pt")
                five_matmuls(pt, x, rt, v, M)
                dst = gq[:, rt : rt + NT + 1 : NT, 0 : 2 * W]
                if rt == 3:
                    nc.vector.tensor_copy(dst, pt[:, :, :])
                else:
                    nc.scalar.copy(dst, pt[:, :, :])
                # per-rt q so the DVE s-chain overlaps later extracts
                nc.vector._custom_dve(
                    sumsq_op,
                    out=q[:, rt : rt + NT + 1 : NT, :],
                    in0=gq[:, rt : rt + NT + 1 : NT, 0:W],
                    in1=gq[:, rt : rt + NT + 1 : NT, W : 2 * W],
                )
                if cb is not None:
                    cb(rt)
            # 7-row tail -> packed tile pages (stage at partition 0, then
            # SBUF->SBUF DMA: compute engines can't address partition 7k)
            r0, M, i0, v = T4
            pt = ps_pool.tile([128, 2, 2 * W], f32, tag="pt")
            five_matmuls(pt, x, NT, v, M)
            stg = st_pool.tile([128, 2, 2 * W], f16, tag="stg")
            nc.scalar.copy(stg[0:7, :, :], pt[0:7, :, :])
            slot = 7 * pair
            for i in (0, 1):
                # issue on the idle GpSimd queue: on Sync these would block
                # behind the T4 extract and stall the next pair's input DMAs
                nc.gpsimd.dma_start(
                    tz[slot : slot + 7, i, 0 : 2 * W], stg[0:7, i, :]
                )
            return gq, q

        def stage_a2(pair, q, recip_on_act):
            """u = 1/q in place (ACT for early pairs, DVE late)."""
            for i in (0, 1):
                sl = slice(i * NT, (i + 1) * NT)
                if recip_on_act:
                    _act_recip(nc, q[:, sl, :], q[:, sl, :])
                else:
                    nc.vector._custom_dve(
                        RECIPROCAL_APPROX_FAST,
                        out=q[:, sl, :],
                        in0=q[:, sl, :],
                        **RECIP_APPROX_FAST_CONSTS,
                    )
            return q

        def stage_b1_pieces(pair, gq, q):
            """inv and n_z sqrts, as 4 individually issuable pieces."""
            inv = iv_pool.tile([128, 2 * NT, W], f16, tag="inv")

            def piece(j):
                i = j // 2
                sl = slice(i * NT, (i + 1) * NT)
                if j % 2 == 0:
                    nc.scalar.activation(
                        inv[:, sl, :], q[:, sl, :], AF.Sqrt, bias=bias0[:, :],
                        scale=16.0 / 63.0,
                    )
                else:
                    nc.scalar.activation(
                        gq[:, sl, 2 * W : 3 * W], q[:, sl, :], AF.Sqrt,
                        bias=bias_nz[:, :], scale=64.0 / 63.0,
                    )

            return inv, piece

        def do_muls(gq, inv, i):
            sl = slice(i * NT, (i + 1) * NT)
            nc.vector.tensor_tensor(
                gq[:, sl, 0:W], gq[:, sl, 0:W], inv[:, sl, :], OP.mult
            )
            nc.vector.tensor_tensor(
                gq[:, sl, W : 2 * W], gq[:, sl, W : 2 * W], inv[:, sl, :],
                OP.mult,
            )

        def stage_b2(pair, gq, inv, scratch, on_act=False, skip_muls=False):
            """in-place muls, then |gen-tgt| sum: fused DVE custom for two
            row tiles, DVE-sub + ACT-abs-accum for the other two (splits
            the reduce across both engines)."""
            if not skip_muls:
                nc.vector.tensor_tensor(
                    gq[:, :, 0:W], gq[:, :, 0:W], inv[:, :, :], OP.mult
                )
                nc.vector.tensor_tensor(
                    gq[:, :, W : 2 * W], gq[:, :, W : 2 * W], inv[:, :, :],
                    OP.mult,
                )
            # pipelined pairs: 3 of 4 reduces on ACT (it has mid-kernel
            # slack); final pair: 1 of 4 (ACT is the tail straggler there)
            order = [3, 0, 1, 2] if on_act else [0, 1, 2, 3]
            for rt in order:
                r0, M, i0, v = ROW_TILES[rt]
                col = pair * NT + rt
                if (rt >= 3) if on_act else True:
                    # half of every pair's reduce runs on ACT: balances the
                    # engines and drains the tail concurrently
                    dpg = scratch[0:M, rt % 2, :]
                    nc.vector.tensor_tensor(
                        dpg, gq[0:M, rt, :], gq[0:M, NT + rt, :], OP.subtract
                    )
                    nc.scalar.activation(
                        dpg, dpg, AF.Abs, bias=bias0[0:M, :],
                        accum_out=accbuf[0:M, col : col + 1],
                    )
                else:
                    nc.vector._custom_dve(
                        subabs_op,
                        out=scratch[0:M, 0, :],
                        in0=gq[0:M, rt, :],
                        in1=gq[0:M, NT + rt, :],
                        accum_out=accbuf[0:M, col : col + 1],
                    )

        def tail_z(scratch):
            """chain for the packed 7-row tail (gen page 0, tgt page 1)."""
            nc.vector._custom_dve(
                sumsq_op, out=qz[0:28, :, :], in0=tz[0:28, :, 0:W],
                in1=tz[0:28, :, W : 2 * W],
            )
            nc.vector._custom_dve(
                RECIPROCAL_APPROX_FAST, out=qz[0:28, :, :], in0=qz[0:28, :, :],
                **RECIP_APPROX_FAST_CONSTS,
            )
            nc.scalar.activation(
                ziv[0:28, :, :], qz[0:28, :, :], AF.Sqrt, bias=bias0[0:28, :],
                scale=16.0 / 63.0,
            )
            nc.scalar.activation(
                tz[0:28, :, 2 * W : 3 * W], qz[0:28, :, :], AF.Sqrt,
                bias=bias_nz[0:28, :], scale=64.0 / 63.0,
            )
            nc.vector.tensor_tensor(
                tz[0:28, :, 0:W], tz[0:28, :, 0:W], ziv[0:28, :, :], OP.mult
            )
            nc.vector.tensor_tensor(
                tz[0:28, :, W : 2 * W], tz[0:28, :, W : 2 * W],
                ziv[0:28, :, :], OP.mult,
            )
            nc.vector._custom_dve(
                subabs_op,
                out=scratch[64:92, 0, :],
                in0=tz[0:28, 0, :],
                in1=tz[0:28, 1, :],
                accum_out=accbuf[0:28, N_ACC_COLS - 1 : N_ACC_COLS],
            )

        scratch = sc_pool.tile([128, 2, 3 * W], f16)
        # software pipeline: A(p) runs ahead of B(p-1); ACT queue sees
        # ext0 ext1 rcp0 rcp1 [table switch] sqrt0 ext2 sqrt1 ext3 ...
        gq0, q0 = stage_a1(0)
        gq1, q1 = stage_a1(1)
        stage_a2(0, q0, False)
        stage_a2(1, q1, False)
        iv0, p0 = stage_b1_pieces(0, gq0, q0)
        for j in range(4):
            p0(j)
        stage_b2(0, gq0, iv0, scratch)
        gq2, q2 = stage_a1(2)
        iv1, p1 = stage_b1_pieces(1, gq1, q1)
        for j in range(4):
            p1(j)
        stage_a2(2, q2, False)
        stage_b2(1, gq1, iv1, scratch)
        gq3, q3 = stage_a1(3)
        iv2, p2 = stage_b1_pieces(2, gq2, q2)
        for j in range(4):
            p2(j)
        stage_a2(3, q3, False)
        stage_b2(2, gq2, iv2, scratch)
        tail_z(scratch)
        iv3, p3 = stage_b1_pieces(3, gq3, q3)
        p3(0)          # inv sqrt, image 0
        do_muls(gq3, iv3, 0)   # muls(i0) run while ACT does nz(i0)
        p3(1)
        p3(2)
        do_muls(gq3, iv3, 1)
        p3(3)
        stage_b2(3, gq3, iv3, scratch, on_act=True, skip_muls=True)

        nc.sync.dma_start(acc_d[:], accbuf[:])


_CACHE = {}


def _get_module():
    if "nc" not in _CACHE:
        from concourse import bacc, tile, mybir

        sumsq_op, subabs_op = _register_ops()
        nc = bacc.Bacc(
            "TRN2",
            target_bir_lowering=False,
            debug=False,
            enable_asserts=True,
            num_devices=N_CORES,
        )
        gen_d = nc.dram_tensor(
            "gen", (PAIRS_PER_CORE, H, W), mybir.dt.float32r, kind="ExternalInput"
        ).ap()
        tgt_d = nc.dram_tensor(
            "tgt", (PAIRS_PER_CORE, H, W), mybir.dt.float32r, kind="ExternalInput"
        ).ap()
        w_d = nc.dram_tensor(
            "w", (128, 12 * 128), mybir.dt.float32r, kind="ExternalInput"
        ).ap()
        acc_d = nc.dram_tensor(
            "acc", (128, N_ACC_COLS), mybir.dt.float32, kind="ExternalOutput"
        ).ap()
        with tile.TileContext(nc) as tc:
            _kernel_body(tc, gen_d, tgt_d, w_d, acc_d, sumsq_op, subabs_op)
        nc.compile()
        _CACHE["nc"] = nc
        _CACHE["w"] = _build_bands_np()
    return _CACHE["nc"], _CACHE["w"]


def _run(generated, target, **spmd_kwargs):
    from concourse import bass_utils

    nc, w = _get_module()
    g = np.ascontiguousarray(np.asarray(generated, np.float32).reshape(TOTAL_B, H, W))
    t = np.ascontiguousarray(np.asarray(target, np.float32).reshape(TOTAL_B, H, W))
    in_maps = [
        {
            "gen": g[c * PAIRS_PER_CORE : (c + 1) * PAIRS_PER_CORE],
            "tgt": t[c * PAIRS_PER_CORE : (c + 1) * PAIRS_PER_CORE],
            "w": w,
        }
        for c in range(N_CORES)
    ]
    return bass_utils.run_bass_kernel_spmd(
        nc, in_maps, core_ids=list(range(N_CORES)), **spmd_kwargs
    )


def kernel(generated, target):
    res = _run(generated, target)
    total = 0.0
    for r in res.results:
        total += float(np.asarray(r["acc"], np.float64).sum())
    return np.float32(total / (TOTAL_B * 3 * H * W))


# revision 52
# speedup vs baseline: 1.0221x; 1.0221x over previous
"""HeightmapNormalsLoss TRN2 kernel, v3.

Data-parallel over 8 NeuronCores: 4 image-pairs per core.

Per image: Sobel gx/gy via TensorEngine band matmuls in f32r (vertical
bands stationary, horizontal taps as shifted column streams of an
edge-padded f32 tile; weights pre-scaled by sqrt(63)), then:

  q   = gx'^2 + gy'^2 + 1          (DVE custom fused op, = 63*s+1)
  u   = 1/q                        (DVE RECIPROCAL_APPROX_FAST; keeping
                                    it off ACT unchokes the extract path)
  inv = sqrt(16/63 * u)            (ACT Sqrt)
  n_z = sqrt(64/63 * u - 1/63)     (ACT Sqrt, written directly)
  n_x = gx'*inv, n_y = gy'*inv     (DVE in-place muls, fp16 2x)
  acc += sum |n_gen - n_tgt|       (pipelined pairs: DVE sub + ACT
                                    Abs-accum; final pair mostly via the
                                    DVE fused sub+abs+reduce op, since
                                    ACT is the straggler in the tail)

Row tiling: 4 full 128-row tiles per image cover rows 0..504; the last
7 rows of all 8 images are packed (via staging + GpSimd-issued SBUF DMA)
into one 2-page tile whose chain runs once in the pipeline tail. PSUM
extraction on ACT Copy (rt3 on DVE for balance); a software pipeline
runs pair p's matmul/extract ahead of pair p-1's sqrt/mul/reduce tail so
every engine queue stays fed. Per-core output: [128, 17] f32 partial
sums; host reduces and divides.
"""
import sys

sys.path.insert(0, "/opt/trn_rl_repo")

import numpy as np

H = W = 512
N_CORES = 8
PAIRS_PER_CORE = 4
TOTAL_B = 32
NT = 4  # full row tiles per image (the 7-row tail is packed separately)
S63 = float(np.sqrt(63.0))

# (out_row_start, M, in_row_start, variant); K = 128 for all tiles.
ROW_TILES = [
    (0, 127, 0, 0),
    (127, 126, 126, 1),
    (253, 126, 252, 1),
    (379, 126, 378, 1),
]
T4 = (505, 7, 384, 2)
N_ACC_COLS = PAIRS_PER_CORE * NT + 1  # 17 (last col = packed 7-row tail)


def _build_bands_np():
    """[128, 12*128] f32: blocks (band*3 + variant); bands sv, -sv, dv, 2dv,
    all scaled by sqrt(63). Variant 2 sits at partitions 120..127 (tile
    loaded from row 384 so K=128 stays in bounds)."""
    mats = {}
    for v, M in ((0, 127), (1, 126), (2, 7)):
        sv = np.zeros((128, 128), np.float64)
        dv = np.zeros((128, 128), np.float64)
        if v == 0:  # first tile: in-row p = image row p; m=0 clamps row -1 -> 0
            sv[0, 0], sv[1, 0] = 3.0, 1.0
            dv[0, 0], dv[1, 0] = 1.0, -1.0
            for m in range(1, M):
                sv[m - 1, m], sv[m, m], sv[m + 1, m] = 1.0, 2.0, 1.0
                dv[m - 1, m], dv[m + 1, m] = 1.0, -1.0
        elif v == 1:  # mid tiles: out r0+m taps partitions m, m+1, m+2
            for m in range(M):
                sv[m, m], sv[m + 1, m], sv[m + 2, m] = 1.0, 2.0, 1.0
                dv[m, m], dv[m + 2, m] = 1.0, -1.0
        else:  # last tile: rows 505..511 from partitions 120..127; clamp row 512
            for m in range(M - 1):
                sv[120 + m, m], sv[121 + m, m], sv[122 + m, m] = 1.0, 2.0, 1.0
                dv[120 + m, m], dv[122 + m, m] = 1.0, -1.0
            m = M - 1
            sv[126, m], sv[127, m] = 1.0, 3.0
            dv[126, m], dv[127, m] = 1.0, -1.0
        mats[(0, v)] = sv
        mats[(1, v)] = -sv
        mats[(2, v)] = dv
        mats[(3, v)] = 2.0 * dv
    w = np.zeros((128, 12 * 128), np.float64)
    for b in range(4):
        for v in range(3):
            w[:, (b * 3 + v) * 128 : (b * 3 + v + 1) * 128] = mats[(b, v)]
    return (w * S63).astype(np.float32)


def _register_ops():
    """Register fused custom DVE ops (runtime append to dve_ops.OPS, sha
    computed self-consistently):
      SUMSQ1_ANT:  out = in0^2 + in1^2 + 1
      SUBABS_ANT:  out = |in0 - in1|, accum_out = sum(out)
    """
    import concourse.dve_ops as dve_ops
    from concourse.dve_spec import Spec, Src0, Src1, One, Zero, maxx, lower
    from concourse.dve_uop import DveOpSpec
    from operator import add

    def reg(name, spec):
        for o in dve_ops.OPS:
            if o.name == name:
                return o
        row = dve_ops._CUSTOM_DVE_ROW_BASE + len(dve_ops.OPS)
        shas = {}
        for ver in ("v3", "v4"):
            uops = lower(spec, ver=ver)
            shas[ver] = DveOpSpec(
                name=name, opcode=row, uops=uops, rd1_en=True
            ).sha(ver)
        op = dve_ops.DveOp(name, spec, subdim=False, uops_sha=shas)
        dve_ops.OPS.append(op)
        dve_ops._SUB_OPCODE_FOR_NAME[name] = row
        dve_ops.CUSTOM_DVE_SPECS[name] = spec
        return op

    def sumsq_ref(in0, in1, s0, s1, imm2):
        return (
            in0.astype(np.float32) ** 2 + in1.astype(np.float32) ** 2 + 1.0
        ).astype(np.float32)

    def subabs_ref(in0, in1, s0, s1, imm2):
        b = np.abs(in0.astype(np.float32) - in1.astype(np.float32)).astype(
            np.float32
        )
        return b, b.reshape(b.shape[0], -1).sum(axis=-1, keepdims=True)

    sumsq = reg(
        "SUMSQ1_ANT", Spec(body=Src0 * Src0 + Src1 * Src1 + One, reference=sumsq_ref)
    )
    subabs = reg(
        "SUBABS_ANT",
        Spec(
            body=maxx(Src0 - Src1, Src1 - Src0),
            accum=add,
            accum_init=Zero,
            reference=subabs_ref,
        ),
    )
    return sumsq, subabs


def _act_recip(nc, out, in_):
    """ACT Reciprocal via direct InstActivation (wrapper blocks it for
    accuracy; 2e-2 tolerance here makes the table fine)."""
    from concourse import mybir

    sc = nc.scalar
    ins = [sc.lower_ap(in_)]
    for arg in (0.0, 1.0, 0.0):  # bias, scale, alpha
        ins.append(mybir.ImmediateValue(dtype=mybir.dt.float32, value=float(arg)))
    return sc.add_instruction(
        mybir.InstActivation(
            name=nc.get_next_instruction_name(),
            func=mybir.ActivationFunctionType.Reciprocal,
            ins=ins,
            outs=[sc.lower_ap(out)],
        )
    )


def _kernel_body(tc, gen_d, tgt_d, w_d, acc_d, sumsq_op, subabs_op):
    from contextlib import ExitStack
    from concourse import mybir
    from concourse.dve_ops import RECIPROCAL_APPROX_FAST, RECIP_APPROX_FAST_CONSTS

    nc = tc.nc
    AF = mybir.ActivationFunctionType
    OP = mybir.AluOpType
    f32 = mybir.dt.float32
    f32r = mybir.dt.float32r
    f16 = mybir.dt.float16

    with ExitStack() as ctx:
        persist = ctx.enter_context(tc.tile_pool(name="persist", bufs=1))
        xp_pool = ctx.enter_context(tc.tile_pool(name="xp", bufs=4))
        ps_pool = ctx.enter_context(tc.tile_pool(name="ps", bufs=2, space="PSUM"))
        gq_pool = ctx.enter_context(tc.tile_pool(name="gq", bufs=2))
        q_pool = ctx.enter_context(tc.tile_pool(name="q", bufs=2))
        iv_pool = ctx.enter_context(tc.tile_pool(name="iv", bufs=2))
        sc_pool = ctx.enter_context(tc.tile_pool(name="sc", bufs=1))
        st_pool = ctx.enter_context(tc.tile_pool(name="st", bufs=2))

        wt = persist.tile([128, 12 * 128], f32r)
        nc.sync.dma_start(wt[:], w_d[:])
        accbuf = persist.tile([128, N_ACC_COLS], f32)
        nc.vector.memset(accbuf[:], 0.0)
        bias0 = persist.tile([128, 1], f32)
        nc.vector.memset(bias0[:], 0.0)
        bias_nz = persist.tile([128, 1], f32)
        nc.vector.memset(bias_nz[:], -1.0 / 63.0)
        # packed tail tile: page 0 = gen, page 1 = tgt; pair p at
        # partitions 7p..7p+6 (written by DMA, which has no partition-
        # alignment restriction; compute reads all start at partition 0)
        tz = persist.tile([128, 2, 3 * W], f16)
        qz = persist.tile([128, 2, W], f16)
        ziv = persist.tile([128, 2, W], f16)

        def w_sl(band, variant, M):
            blk = (band * 3 + variant) * 128
            return wt[0:128, blk : blk + M]

        def five_matmuls(pt, x, rt, v, M):
            """x: [128, 2img, NT+1, W+2]; pt: [128, 2img, 2W] psum; the two
            images' five streams interleave per stationary (one LDW each)."""
            sv, nsv = w_sl(0, v, M), w_sl(1, v, M)
            dv, dv2 = w_sl(2, v, M), w_sl(3, v, M)
            for i in (0, 1):
                gx = pt[0:M, i, 0:W]
                nc.tensor.matmul(
                    gx, sv, x[:, i, rt, 0:W], start=True, stop=False
                )
                nc.tensor.matmul(
                    gx, nsv, x[:, i, rt, 2 : W + 2], start=False, stop=True
                )
            for i in (0, 1):
                gy = pt[0:M, i, W : 2 * W]
                nc.tensor.matmul(
                    gy, dv, x[:, i, rt, 0:W], start=True, stop=False
                )
                nc.tensor.matmul(
                    gy, dv, x[:, i, rt, 2 : W + 2], start=False, stop=False
                )
                nc.tensor.matmul(
                    gy, dv2, x[:, i, rt, 1 : W + 1], start=False, stop=True
                )

        def stage_a1(pair, cb=None):
            """DMA, matmuls, extract -> gq (4 row tiles) + tz (7-row tail).
            cb(rt), if given, issues one piece of the previous pair's sqrt
            work after each row-tile extract (keeps ACT from blocking PE)."""
            xp = xp_pool.tile([128, 2, NT + 1, W + 2], f32r, tag="xp")
            for rt, (r0, M, i0, v) in enumerate(ROW_TILES + [T4]):
                for imi, src in ((0, gen_d), (1, tgt_d)):
                    nc.sync.dma_start(
                        xp[0:128, imi, rt, 1 : W + 1], src[pair, i0 : i0 + 128, :]
                    )
                    if pair == 0:
                        # pair 0: per-tile pads so the first matmuls start
                        # right after their own DMA (startup latency)
                        nc.vector.tensor_copy(
                            xp[:, imi, rt, 0:1], xp[:, imi, rt, 1:2]
                        )
                        nc.vector.tensor_copy(
                            xp[:, imi, rt, W + 1 : W + 2],
                            xp[:, imi, rt, W : W + 1],
                        )
            if pair > 0:
                # later pairs prefetch ahead: two batched pads per image
                for imi in (0, 1):
                    nc.vector.tensor_copy(
                        xp[:, imi, :, 0:1], xp[:, imi, :, 1:2]
                    )
                    nc.vector.tensor_copy(
                        xp[:, imi, :, W + 1 : W + 2], xp[:, imi, :, W : W + 1]
                    )

            gq = gq_pool.tile([128, 2 * NT, 3 * W], f16, tag="gq")
            q = q_pool.tile([128, 2 * NT, W], f16, tag="q")
            x = xp[0:128, :, :, :]
            for rt, (r0, M, i0, v) in enumerate(ROW_TILES):
                pt = ps_pool.tile([128, 2, 2 * W], f32, tag="pt")
                five_matmuls(pt, x, rt, v, M)
                dst = gq[:, rt : rt + NT + 1 : NT, 0 : 2 * W]
                if rt == 3:
                    nc.vector.tensor_copy(dst, pt[:, :, :])
                else:
                    nc.scalar.copy(dst, pt[:, :, :])
                # per-rt q so the DVE s-chain overlaps later extracts
                nc.vector._custom_dve(
                    sumsq_op,
                    out=q[:, rt : rt + NT + 1 : NT, :],
                    in0=gq[:, rt : rt + NT + 1 : NT, 0:W],
                    in1=gq[:, rt : rt + NT + 1 : NT, W : 2 * W],
                )
                if cb is not None:
                    cb(rt)
            # 7-row tail -> packed tile pages (stage at partition 0, then
            # SBUF->SBUF DMA: compute engines can't address partition 7k)
            r0, M, i0, v = T4
            pt = ps_pool.tile([128, 2, 2 * W], f32, tag="pt")
            five_matmuls(pt, x, NT, v, M)
            stg = st_pool.tile([128, 2, 2 * W], f16, tag="stg")
            nc.scalar.copy(stg[0:7, :, :], pt[0:7, :, :])
            slot = 7 * pair
            for i in (0, 1):
                # issue on the idle GpSimd queue: on Sync these would block
                # behind the T4 extract and stall the next pair's input DMAs
                nc.gpsimd.dma_start(
                    tz[slot : slot + 7, i, 0 : 2 * W], stg[0:7, i, :]
                )
            return gq, q

        def stage_a2(pair, q, recip_on_act):
            """u = 1/q in place (ACT for early pairs, DVE late)."""
            for i in (0, 1):
                sl = slice(i * NT, (i + 1) * NT)
                if recip_on_act:
                    _act_recip(nc, q[:, sl, :], q[:, sl, :])
                else:
                    nc.vector._custom_dve(
                        RECIPROCAL_APPROX_FAST,
                        out=q[:, sl, :],
                        in0=q[:, sl, :],
                        **RECIP_APPROX_FAST_CONSTS,
                    )
            return q

        def stage_b1_pieces(pair, gq, q):
            """inv and n_z sqrts, as 4 individually issuable pieces."""
            inv = iv_pool.tile([128, 2 * NT, W], f16, tag="inv")

            def piece(j):
                i = j // 2
                sl = slice(i * NT, (i + 1) * NT)
                if j % 2 == 0:
                    nc.scalar.activation(
                        inv[:, sl, :], q[:, sl, :], AF.Sqrt, bias=bias0[:, :],
                        scale=16.0 / 63.0,
                    )
                else:
                    nc.scalar.activation(
                        gq[:, sl, 2 * W : 3 * W], q[:, sl, :], AF.Sqrt,
                        bias=bias_nz[:, :], scale=64.0 / 63.0,
                    )

            return inv, piece

        def do_muls(gq, inv, i):
            sl = slice(i * NT, (i + 1) * NT)
            nc.vector.tensor_tensor(
                gq[:, sl, 0:W], gq[:, sl, 0:W], inv[:, sl, :], OP.mult
            )
            nc.vector.tensor_tensor(
                gq[:, sl, W : 2 * W], gq[:, sl, W : 2 * W], inv[:, sl, :],
                OP.mult,
            )

        def stage_b2(pair, gq, inv, scratch, on_act=False, skip_muls=False):
            """in-place muls, then |gen-tgt| sum: fused DVE custom for two
            row tiles, DVE-sub + ACT-abs-accum for the other two (splits
            the reduce across both engines)."""
            if not skip_muls:
                for i in (0, 1):
                    do_muls(gq, inv, i)
            # pipelined pairs: 3 of 4 reduces on ACT (it has mid-kernel
            # slack); final pair: 1 of 4 (ACT is the tail straggler there)
            order = [3, 0, 1, 2] if on_act else [0, 1, 2, 3]
            for rt in order:
                r0, M, i0, v = ROW_TILES[rt]
                col = pair * NT + rt
                if (rt >= 3) if on_act else True:
                    # half of every pair's reduce runs on ACT: balances the
                    # engines and drains the tail concurrently
                    dpg = scratch[0:M, rt % 2, :]
                    nc.vector.tensor_tensor(
                        dpg, gq[0:M, rt, :], gq[0:M, NT + rt, :], OP.subtract
                    )
                    nc.scalar.activation(
                        dpg, dpg, AF.Abs, bias=bias0[0:M, :],
                        accum_out=accbuf[0:M, col : col + 1],
                    )
                else:
                    nc.vector._custom_dve(
                        subabs_op,
                        out=scratch[0:M, 0, :],
                        in0=gq[0:M, rt, :],
                        in1=gq[0:M, NT + rt, :],
                        accum_out=accbuf[0:M, col : col + 1],
                    )

        def tail_z(scratch):
            """chain for the packed 7-row tail (gen page 0, tgt page 1)."""
            nc.vector._custom_dve(
                sumsq_op, out=qz[0:28, :, :], in0=tz[0:28, :, 0:W],
                in1=tz[0:28, :, W : 2 * W],
            )
            nc.vector._custom_dve(
                RECIPROCAL_APPROX_FAST, out=qz[0:28, :, :], in0=qz[0:28, :, :],
                **RECIP_APPROX_FAST_CONSTS,
            )
            nc.scalar.activation(
                ziv[0:28, :, :], qz[0:28, :, :], AF.Sqrt, bias=bias0[0:28, :],
                scale=16.0 / 63.0,
            )
            nc.scalar.activation(
                tz[0:28, :, 2 * W : 3 * W], qz[0:28, :, :], AF.Sqrt,
                bias=bias_nz[0:28, :], scale=64.0 / 63.0,
            )
            nc.vector.tensor_tensor(
                tz[0:28, :, 0:W], tz[0:28, :, 0:W], ziv[0:28, :, :], OP.mult
            )
            nc.vector.tensor_tensor(
                tz[0:28, :, W : 2 * W], tz[0:28, :, W : 2 * W],
                ziv[0:28, :, :], OP.mult,
            )
            nc.vector._custom_dve(
                subabs_op,
                out=scratch[64:92, 0, :],
                in0=tz[0:28, 0, :],
                in1=tz[0:28, 1, :],
                accum_out=accbuf[0:28, N_ACC_COLS - 1 : N_ACC_COLS],
            )

        scratch = sc_pool.tile([128, 2, 3 * W], f16)
        # software pipeline: A(p) runs ahead of B(p-1); ACT queue sees
        # ext0 ext1 rcp0 rcp1 [table switch] sqrt0 ext2 sqrt1 ext3 ...
        gq0, q0 = stage_a1(0)
        gq1, q1 = stage_a1(1)
        stage_a2(0, q0, False)
        stage_a2(1, q1, False)
        iv0, p0 = stage_b1_pieces(0, gq0, q0)
        for j in range(4):
            p0(j)
        stage_b2(0, gq0, iv0, scratch)
        gq2, q2 = stage_a1(2)
        iv1, p1 = stage_b1_pieces(1, gq1, q1)
        for j in range(4):
            p1(j)
        stage_a2(2, q2, False)
        stage_b2(1, gq1, iv1, scratch)
        gq3, q3 = stage_a1(3)
        iv2, p2 = stage_b1_pieces(2, gq2, q2)
        for j in range(4):
            p2(j)
        stage_a2(3, q3, False)
        stage_b2(2, gq2, iv2, scratch)
        tail_z(scratch)
        iv3, p3 = stage_b1_pieces(3, gq3, q3)
        p3(0)          # inv sqrt, image 0
        do_muls(gq3, iv3, 0)   # muls(i0) run while ACT does nz(i0)
        p3(1)
        p3(2)
        do_muls(gq3, iv3, 1)
        p3(3)
        stage_b2(3, gq3, iv3, scratch, on_act=True, skip_muls=True)

        nc.sync.dma_start(acc_d[:], accbuf[:])


_CACHE = {}


def _get_module():
    if "nc" not in _CACHE:
        from concourse import bacc, tile, mybir

        sumsq_op, subabs_op = _register_ops()
        nc = bacc.Bacc(
            "TRN2",
            target_bir_lowering=False,
            debug=False,
            enable_asserts=True,
            num_devices=N_CORES,
        )
        gen_d = nc.dram_tensor(
            "gen", (PAIRS_PER_CORE, H, W), mybir.dt.float32r, kind="ExternalInput"
        ).ap()
        tgt_d = nc.dram_tensor(
            "tgt", (PAIRS_PER_CORE, H, W), mybir.dt.float32r, kind="ExternalInput"
        ).ap()
        w_d = nc.dram_tensor(
            "w", (128, 12 * 128), mybir.dt.float32r, kind="ExternalInput"
        ).ap()
        acc_d = nc.dram_tensor(
            "acc", (128, N_ACC_COLS), mybir.dt.float32, kind="ExternalOutput"
        ).ap()
        with tile.TileContext(nc) as tc:
            _kernel_body(tc, gen_d, tgt_d, w_d, acc_d, sumsq_op, subabs_op)
        nc.compile()
        _CACHE["nc"] = nc
        _CACHE["w"] = _build_bands_np()
    return _CACHE["nc"], _CACHE["w"]


def _run(generated, target, **spmd_kwargs):
    from concourse import bass_utils

    nc, w = _get_module()
    g = np.ascontiguousarray(np.asarray(generated, np.float32).reshape(TOTAL_B, H, W))
    t = np.ascontiguousarray(np.asarray(target, np.float32).reshape(TOTAL_B, H, W))
    in_maps = [
        {
            "gen": g[c * PAIRS_PER_CORE : (c + 1) * PAIRS_PER_CORE],
            "tgt": t[c * PAIRS_PER_CORE : (c + 1) * PAIRS_PER_CORE],
            "w": w,
        }
        for c in range(N_CORES)
    ]
    return bass_utils.run_bass_kernel_spmd(
        nc, in_maps, core_ids=list(range(N_CORES)), **spmd_kwargs
    )


def kernel(generated, target):
    res = _run(generated, target)
    total = 0.0
    for r in res.results:
        total += float(np.asarray(r["acc"], np.float64).sum())
    return np.float32(total / (TOTAL_B * 3 * H * W))
